# revision 55
# baseline (speedup 1.0000x reference)
"""Sparse diag-masked multi-head attention layer on 8 trn2 cores.

Sharding: core = b*4 + g  (b in 0..1 batches, g in 0..3 head-groups).
Each core computes heads 4g..4g+3 of batch b. Head-group g has band
offset off = 2**g: visible(q, s) <=> s >= q + off  OR  s == L-1.

v3 design (fp16 activations, transposed AV):
  qT, kT   [e 128, ec 2, L]  sbuf fp16    (e on partitions, ec = head pair)
  scoresT  [s 128, 2h, 512]  psum f32 = kT_chunk.T @ qT_chunk (contract e=64)
  pt       [s 128, 4h, nv]   sbuf fp16 = exp(0.125 * scores), band-masked
  AV transposed: av[q 128, t=qk*4+h, e 64] psum += pt_tile.T @ v  (contract s)
     + den[q, t] psum += pt_tile.T @ ones   (1-col matmuls)
     psum zero-region (bank) shared by 8 av tiles: first tile of each bank
     issues start=True (clears the whole bank's has_written bits); the rest
     overwrite into pending-zero bytes; stop on the bank's last touch.
  normalize: attn[q, t, e] = av * reciprocal(den)  (DVE, per-partition scalars)
  aT [e, ec, q] = DMA-transpose(attn)   (XBAR, fp16)
  outT [n, q] psum = woT_chunk.T @ aT_chunk  (contract e=256)
Host: out[b] = sum_g outT(b,g).T + (Wo @ bv + bo)
(bk/bq applied on device during psum->sbuf copies; bv/bo folded on host.)

Schedule: k/q projections for the first score tiles run ec-split so the
first exp lands ~9us in; v/k/q projection chunks and the previous j's
out-projection are woven into the attention js-loops as PE filler.
"""
import sys

sys.path.insert(0, "/opt/trn_rl_repo")

import numpy as np

import concourse.bacc as bacc
import concourse.bass as bass
import concourse.mybir as mybir
import concourse.tile as tile

P = 128
L = 2048
D = 1024
EPC = 256  # head-dims per core (4 heads x 64)
EC = 2  # e-chunks of 128
HPC = 4  # heads per core
NQ = 512  # q-chunk width
NJ = L // NQ  # 4
NSB = L // P  # 16 s-blocks
NDC = D // P  # 8 d-chunks
SCALE = 0.125  # 1/sqrt(64)

F32 = mybir.dt.float32
F16 = mybir.dt.float16
X_DT = F16


def build_nc():
    nc = bacc.Bacc("TRN2", target_bir_lowering=False, debug=False)

    xq = nc.dram_tensor("xqT", [D, L], X_DT, kind="ExternalInput")
    xk = nc.dram_tensor("xkT", [D, L], X_DT, kind="ExternalInput")
    xv = nc.dram_tensor("xvT", [D, L], X_DT, kind="ExternalInput")
    wq = nc.dram_tensor("wqT", [D, EPC], X_DT, kind="ExternalInput")
    wk = nc.dram_tensor("wkT", [D, EPC], X_DT, kind="ExternalInput")
    wv = nc.dram_tensor("wvT", [D, EPC], X_DT, kind="ExternalInput")
    wo = nc.dram_tensor("woT", [EPC, D], X_DT, kind="ExternalInput")
    bqk = nc.dram_tensor("bqk", [2, EC, P], F32, kind="ExternalInput")
    ramps = nc.dram_tensor("ramps", [2, P], F32, kind="ExternalInput")
    out = nc.dram_tensor("outT", [D, L], X_DT, kind="ExternalOutput")

    with tile.TileContext(nc) as tc:
        with (
            tc.tile_pool(name="consts", bufs=1) as consts,
            tc.tile_pool(name="acts", bufs=1) as acts,
            tc.tile_pool(name="xp", bufs=12) as xp,
            tc.tile_pool(name="ptp", bufs=8) as ptp,
            tc.tile_pool(name="attnp", bufs=3) as attnp,
            tc.tile_pool(name="rdp", bufs=2) as rdp,
            tc.tile_pool(name="ttp", bufs=4) as ttp,
            tc.tile_pool(name="osbp", bufs=2) as osbp,
            tc.tile_pool(name="atp", bufs=3) as atp,
            tc.tile_pool(name="scp", bufs=2, space=bass.MemorySpace.PSUM) as scp,
            tc.tile_pool(name="avp", bufs=1, space=bass.MemorySpace.PSUM) as avp,
            tc.tile_pool(name="vpp", bufs=1, space=bass.MemorySpace.PSUM) as vpp,
        ):
            # ---- constants / weights ----
            wk_sb = consts.tile([P, NDC, EPC], X_DT, tag="wk")
            wq_sb = consts.tile([P, NDC, EPC], X_DT, tag="wq")
            wv_sb = consts.tile([P, NDC, EPC], X_DT, tag="wv")
            wo_sb = consts.tile([P, EC, D], X_DT, tag="wo")
            bqk_sb = consts.tile([P, 2, EC], F32, tag="bqk")
            ramps_sb = consts.tile([P, 2], F32, tag="ramps")
            ones_col = consts.tile([P, 1], X_DT, tag="ones_col")
            junk = consts.tile([P, EPC], X_DT, tag="junk")
            c0i = consts.tile([P, NQ], mybir.dt.int32, tag="c0i")
            c0s = consts.tile([P, NQ], X_DT, tag="c0s")

            # ---- persistent activations ----
            qT = acts.tile([P, EC, L], X_DT, tag="qT")
            kT = acts.tile([P, EC, L], X_DT, tag="kT")
            vpk = acts.tile([P, NSB, HPC, 64], X_DT, tag="vpk")
            aT_tiles = {}

            # ---- small on-device consts (issued early; cheap) ----
            nc.vector.memset(junk[:], 0.125)
            nc.vector.memset(ones_col[:], 1.0)
            nc.gpsimd.iota(c0i[:], pattern=[[1, NQ]], base=0, channel_multiplier=0)
            nc.vector.tensor_copy(c0s[:], c0i[:])

            # ---- PE warmup (junk matmuls; output never read) ----
            wu = vpp.tile([P, EC, EPC], F32, tag="vp", name="warm")
            for _ in range(10):
                nc.tensor.matmul(wu[:, 0, :], junk[:, 0:P], junk[:, 0:EPC], start=True, stop=True)

            # ---- x-tile DMAs ([P, dc, q] per 512-col chunk) ----
            xk_t = [None] * NJ
            xq_t = [None] * NJ
            xv_t = [None] * NJ

            def load_x(dst_list, dram, c, tag, halves=False):
                t = xp.tile([P, NDC, NQ], X_DT, tag=tag, name=f"{tag}{c}", bufs=4)
                if halves:
                    for h in range(2):
                        nc.sync.dma_start(
                            t[:, 4 * h : 4 * h + 4, :],
                            dram[
                                4 * h * P : 4 * (h + 1) * P, c * NQ : (c + 1) * NQ
                            ].rearrange("(dc p) q -> p dc q", p=P),
                        )
                else:
                    nc.sync.dma_start(
                        t[:],
                        dram[:, c * NQ : (c + 1) * NQ].rearrange(
                            "(dc p) q -> p dc q", p=P
                        ),
                    )
                dst_list[c] = t

            # DMA issue order = arrival priority.
            def load_x_halves(dst_list, dram, c, tag):
                t = xp.tile([P, NDC, NQ], X_DT, tag=tag, name=f"{tag}{c}", bufs=4)
                for h in range(2):
                    nc.sync.dma_start(
                        t[:, 4 * h : 4 * h + 4, :],
                        dram[
                            4 * h * P : 4 * (h + 1) * P, c * NQ : (c + 1) * NQ
                        ].rearrange("(dc p) q -> p dc q", p=P),
                    )
                dst_list[c] = t

            nc.sync.dma_start(wk_sb[:], wk.rearrange("(dc p) e -> p dc e", p=P))
            load_x_halves(xk_t, xk, 3, "xk")
            nc.sync.dma_start(bqk_sb[:], bqk.rearrange("t c p -> p t c"))
            nc.sync.dma_start(ramps_sb[:], ramps.rearrange("t p -> p t"))
            nc.sync.dma_start(wq_sb[:], wq.rearrange("(dc p) e -> p dc e", p=P))
            load_x_halves(xq_t, xq, 0, "xq")
            nc.sync.dma_start(wv_sb[:], wv.rearrange("(dc p) e -> p dc e", p=P))
            load_x(xv_t, xv, 3, "xv")
            load_x(xk_t, xk, 2, "xk")
            load_x(xv_t, xv, 2, "xv")
            load_x(xk_t, xk, 1, "xk")
            load_x(xq_t, xq, 1, "xq")
            load_x(xv_t, xv, 1, "xv")
            load_x(xk_t, xk, 0, "xk")
            load_x(xv_t, xv, 0, "xv")
            load_x(xq_t, xq, 2, "xq")
            load_x(xq_t, xq, 3, "xq")
            nc.sync.dma_start(wo_sb[:], wo.rearrange("(c p) n -> p c n", p=P))

            # ================= emitters =================
            def emit_kqproj(c, wsb, xc, dst, bias_row, ecs=(0, 1)):
                """Project x chunk c -> dst[:, ec, c*NQ:(c+1)*NQ] (+bias)."""
                slot = scp.tile([P, EC, NQ], F32, tag="sc", name=f"pj{c}")
                for ec in ecs:
                    for dc in range(NDC):
                        nc.tensor.matmul(
                            slot[:, ec, :],
                            wsb[:, dc, ec * P : (ec + 1) * P],
                            xc[:, dc, :],
                            start=(dc == 0),
                            stop=(dc == NDC - 1),
                        )
                for ec in ecs:
                    nc.vector.tensor_scalar_add(
                        dst[:, ec, c * NQ : (c + 1) * NQ],
                        slot[:, ec, :],
                        bqk_sb[:, bias_row, ec : ec + 1],
                    )

            def emit_vproj(pair):
                """Project v s-blocks (2*pair, 2*pair+1) -> vpk."""
                sb0 = 2 * pair
                c, base = sb0 // 4, (sb0 % 4) * P
                xc = xv_t[c]
                slot = vpp.tile([P, EC, EPC], F32, tag="vp", name=f"v{pair}")
                for i in range(2):
                    for dc in range(NDC):
                        nc.tensor.matmul(
                            slot[:, i, :],
                            xc[:, dc, base + i * P : base + (i + 1) * P],
                            wv_sb[:, dc, :],
                            start=(dc == 0),
                            stop=(dc == NDC - 1),
                        )
                nc.vector.tensor_copy(
                    vpk[:, sb0 : sb0 + 2, :, :],
                    slot[:].rearrange("p i (h e) -> p i h e", e=64),
                )

            def emit_scores_pr(j, js, pr, pt, tt):
                """One head-pair of scores + exp + mask for (j, js)."""
                q0 = j * NQ
                s0 = js * P
                k = js - 4 * j
                nv = min(NQ, (k + 1) * P)
                masked = k <= 4
                sp = scp.tile([P, EC, NQ], F32, tag="sc", name=f"sc{js}")
                for i in range(2):
                    nc.tensor.matmul(
                        sp[:, i, 0:nv],
                        kT[i * 64 : i * 64 + 64, pr, s0 : s0 + P],
                        qT[i * 64 : i * 64 + 64, pr, q0 : q0 + nv],
                        start=True,
                        stop=True,
                    )
                nc.scalar.activation(
                    pt[:, 2 * pr : 2 * pr + 2, 0:nv],
                    sp[:, :, 0:nv],
                    mybir.ActivationFunctionType.Exp,
                    scale=SCALE,
                )
                if masked:
                    m0 = max(0, k * P - 8)
                    nc.vector.scalar_tensor_tensor(
                        pt[:, 2 * pr : 2 * pr + 2, m0:nv],
                        c0s[:, None, m0:nv].broadcast_to([P, 2, nv - m0]),
                        tt[:],
                        pt[:, 2 * pr : 2 * pr + 2, m0:nv],
                        op0=mybir.AluOpType.is_le,
                        op1=mybir.AluOpType.mult,
                    )

            def make_tt(j, js):
                k = js - 4 * j
                if k > 4:
                    return None
                tt = ttp.tile([P, 1], F32, tag="t", name="t")
                ramp = ramps_sb[:, 1:2] if js == NSB - 1 else ramps_sb[:, 0:1]
                nc.vector.tensor_scalar_add(tt[:], ramp, float(js * P - j * NQ))
                return tt

            def emit_scores(j, js):
                pt = ptp.tile([P, HPC, NQ], X_DT, tag="pt", name=f"pt{js}")
                tt = make_tt(j, js)
                for pr in range(2):
                    emit_scores_pr(j, js, pr, pt, tt)
                return pt

            def emit_av(j, js, pt, av, dens):
                """Accumulate av[q, t, e] += pt_tile.T @ v ; den += pt.T @ 1.

                den is split per av-bank (qk pair) so each half can be read
                while the other bank is still accumulating."""
                k = js - 4 * j
                nv = min(NQ, (k + 1) * P)
                nqk = nv // P
                first = js == NSB - 1
                for qk in range(nqk):
                    for h in range(HPC):
                        # t-order (ec, qk, h%2) so the whole attn block
                        # transposes to aT[p, ec, qk, q'] in ONE XBAR DMA
                        t = (h // 2) * 8 + qk * 2 + (h % 2)
                        lhsT = pt[:, h, qk * P : (qk + 1) * P]
                        nc.tensor.matmul(
                            av[:, t, :],
                            lhsT,
                            vpk[:, js, h, :],
                            start=(first and qk == 0 and h % 2 == 0),
                            stop=(k == 0 and qk == 0 and h % 2 == 1),
                            skip_group_check=True,
                        )
                        nc.tensor.matmul(
                            dens[:, t : t + 1],
                            lhsT,
                            ones_col[:],
                            start=(first and h == 0 and qk == 0),
                            stop=(k == 0 and qk == 0 and h == HPC - 1),
                            skip_group_check=True,
                        )

            def emit_norm(j, av, dens, attn):
                """attn[q, t, e] = av * 1/den ; one XBAR transpose into aT_j."""
                rd = rdp.tile([P, NSB], F32, tag="rd", name="rd")
                nc.vector.reciprocal(rd[:], dens[:])
                for hb in range(2):
                    t0 = hb * 8
                    nc.vector.tensor_mul(
                        attn[:, t0 : t0 + 8, :],
                        av[:, t0 : t0 + 8, :],
                        rd[:, t0 : t0 + 8, None].broadcast_to([P, 8, 64]),
                    )
                aT_j = atp.tile([P, EC, 4, P], X_DT, tag="aT", name=f"aT{j}")
                aT_tiles[j] = aT_j
                nc.sync.dma_start(
                    aT_j[:],
                    attn[:].rearrange("p t e -> p (t e)"),
                    transpose=True,
                )

            def emit_outproj_np2(j, np2, osb, act=False, by_qk=False):
                """Out-projection for n-pair np2 of q-chunk j into osb.

                by_qk: 128-col matmuls chasing per-qk transposes; the psum
                bank is shared by the 4 qk groups via pending-zero."""
                aT_j = aT_tiles[j]
                ops = scp.tile([P, EC, NQ], F32, tag="sc", name=f"op{np2}")
                if by_qk:
                    for qk in range(4):
                        for i in range(2):
                            n = np2 * 2 + i
                            for c in range(EC):
                                nc.tensor.matmul(
                                    ops[:, i, qk * P : (qk + 1) * P],
                                    wo_sb[:, c, n * P : (n + 1) * P],
                                    aT_j[:, c, qk, :],
                                    start=(qk == 0 and c == 0),
                                    stop=(qk == 3 and c == EC - 1),
                                    skip_group_check=True,
                                )
                else:
                    for i in range(2):
                        n = np2 * 2 + i
                        for c in range(EC):
                            nc.tensor.matmul(
                                ops[:, i, :],
                                wo_sb[:, c, n * P : (n + 1) * P],
                                aT_j[:, c, :, :].rearrange("p a q -> p (a q)"),
                                start=(c == 0),
                                stop=(c == EC - 1),
                            )
                if act:
                    nc.scalar.copy(osb[:, np2, :, :], ops[:])
                else:
                    nc.vector.tensor_copy(osb[:, np2, :, :], ops[:])

            def emit_outproj_half(j, np2, osb, hb, act=False):
                """Half-width out-projection (qk pair hb) for n-pair np2."""
                aT_j = aT_tiles[j]
                ops = scp.tile([P, EC, NQ], F32, tag="sc", name=f"oph{np2}")
                for i in range(2):
                    n = np2 * 2 + i
                    for qk in (2 * hb, 2 * hb + 1):
                        for c in range(EC):
                            nc.tensor.matmul(
                                ops[:, i, qk * P : (qk + 1) * P],
                                wo_sb[:, c, n * P : (n + 1) * P],
                                aT_j[:, c, qk, :],
                                start=(qk == 2 * hb and c == 0),
                                stop=(qk == 2 * hb + 1 and c == EC - 1),
                                skip_group_check=True,
                            )
                h0 = 2 * hb * P
                if act:
                    nc.scalar.copy(
                        osb[:, np2, :, h0 : h0 + 2 * P], ops[:, :, h0 : h0 + 2 * P]
                    )
                else:
                    nc.vector.tensor_copy(
                        osb[:, np2, :, h0 : h0 + 2 * P], ops[:, :, h0 : h0 + 2 * P]
                    )

            def emit_store(j, osb, half=None):
                q0 = j * NQ
                if half is None:
                    nc.gpsimd.dma_start(
                        out[:, q0 : q0 + NQ].rearrange(
                            "(np i p) q -> p np i q", i=2, p=P
                        ),
                        osb[:],
                    )
                else:
                    # row-pair split: np2 {0,1} or {2,3} (disjoint from the
                    # other pair's osb writes)
                    r0 = half * 2 * EC * P
                    nc.gpsimd.dma_start(
                        out[r0 : r0 + 2 * EC * P, q0 : q0 + NQ].rearrange(
                            "(np i p) q -> p np i q", i=2, p=P
                        ),
                        osb[:, 2 * half : 2 * half + 2, :, :],
                    )

            def emit_warm(n, dep=None):
                """Junk matmuls to keep the PE p-state warm. `dep` (an sbuf
                f16 AP) pins them in schedule order via a read dependency —
                without it Tile hoists them to an arbitrary slot."""
                w = vpp.tile([P, EC, EPC], F32, tag="vp", name="warm2")
                lhs = junk[:, 0:P] if dep is None else dep[:, 0, 0:P]
                rhs = junk[:, 0:P] if dep is None else dep[:, 0, 0:P]
                for _ in range(n):
                    nc.tensor.matmul(w[:, 0, 0:P], lhs, rhs, start=True, stop=True)

            # ================= schedule =================
            pre_pts = {}

            def emit_pre_scores(j, js):
                pre_pts[(j, js)] = emit_scores(j, js)

            # Head: ec-split k3/q0 so the first exp lands early.
            emit_kqproj(3, wk_sb, xk_t[3], kT, 1, ecs=(0,))
            emit_kqproj(0, wq_sb, xq_t[0], qT, 0, ecs=(0,))
            pt15 = ptp.tile([P, HPC, NQ], X_DT, tag="pt", name="pt15")
            tt15 = make_tt(0, NSB - 1)
            emit_scores_pr(0, NSB - 1, 0, pt15, tt15)
            emit_kqproj(3, wk_sb, xk_t[3], kT, 1, ecs=(1,))
            emit_kqproj(0, wq_sb, xq_t[0], qT, 0, ecs=(1,))
            emit_scores_pr(0, NSB - 1, 1, pt15, tt15)
            pre_pts[(0, NSB - 1)] = pt15

            # filler work injected into the j-loops, keyed by (j, js)
            fillers = {
                (0, 14): [lambda: emit_vproj(7)],
                (0, 13): [lambda: emit_vproj(6),
                          lambda: emit_kqproj(2, wk_sb, xk_t[2], kT, 1)],
                (0, 12): [lambda: emit_vproj(5)],
                (0, 11): [lambda: emit_vproj(4)],
                (0, 10): [lambda: emit_kqproj(1, wk_sb, xk_t[1], kT, 1)],
                (0, 9): [lambda: emit_vproj(3)],
                (0, 8): [lambda: emit_vproj(2),
                         lambda: emit_kqproj(1, wq_sb, xq_t[1], qT, 0)],
                (0, 7): [lambda: emit_kqproj(0, wk_sb, xk_t[0], kT, 1)],
                (0, 6): [lambda: emit_pre_scores(1, 15)],
                (0, 5): [lambda: emit_vproj(1)],
                (0, 4): [lambda: emit_pre_scores(1, 14)],
                (0, 3): [lambda: emit_vproj(0)],
                (0, 2): [lambda: emit_pre_scores(1, 13)],
                (1, 13): [lambda: emit_kqproj(2, wq_sb, xq_t[2], qT, 0)],
                (1, 6): [lambda: emit_pre_scores(2, 15)],
                (1, 5): [lambda: emit_pre_scores(2, 14)],
                (1, 4): [lambda: emit_pre_scores(2, 13)],
                (2, 13): [lambda: emit_kqproj(3, wq_sb, xq_t[3], qT, 0)],
                (2, 9): [lambda: emit_pre_scores(3, 15)],
                (2, 8): [lambda: emit_pre_scores(3, 14)],
            }

            def make_outproj_fillers(jprev, j, js_list):
                """Spread outproj(jprev) + store over j's loop."""
                osb = osbp.tile([P, 4, EC, NQ], X_DT, tag="osb", name=f"osb{jprev}")
                if j == NJ - 1:
                    slots = [12, 12, 12, 12]
                else:
                    slots = (js_list[3:] + [js_list[-1]] * 4)[:4]
                act = jprev >= 2
                for idx, js in enumerate(slots):
                    fillers.setdefault((j, js), []).append(
                        lambda np2=idx: emit_outproj_np2(jprev, np2, osb, act=act)
                    )
                fillers.setdefault((j, slots[3]), []).append(
                    lambda: emit_store(jprev, osb)
                )

            osb3 = [None]
            carry = [None]

            def finalize(c):
                jf, rest, avf, densf, attnf = c
                while rest:
                    js_f, pt_f = rest.pop(0)
                    emit_av(jf, js_f, pt_f, avf, densf)
                emit_norm(jf, avf, densf, attnf)

            last_pt = None
            for j in range(NJ):
                js_list = list(range(NSB - 1, 4 * j - 1, -1))
                if j > 0:
                    make_outproj_fillers(j - 1, j, js_list)
                if j == NJ - 1:
                    osb3[0] = osbp.tile([P, 4, EC, NQ], X_DT, tag="osb", name="osb3")
                av = dens = attn = None
                pend = []
                last_js = js_list[-1]
                for js in js_list:
                    pre = pre_pts.pop((j, js), None)
                    pt = pre if pre is not None else emit_scores(j, js)
                    pend.append((js, pt))
                    if av is None and pre is None:
                        # first live scores emitted: finalize the previous j
                        # (its trailing AVs + norm) behind them, then claim
                        # the av psum tiles
                        if carry[0] is not None:
                            finalize(carry[0])
                            carry[0] = None
                        av = avp.tile([P, NSB, 64], F32, tag="av", name=f"av{j}")
                        dens = avp.tile([P, NSB], F32, tag="den", name=f"den{j}")
                        attn = attnp.tile([P, NSB, 64], X_DT, tag="attn", name=f"at{j}")
                    if js == last_js:
                        # flush all but the current so fillers (prev outproj)
                        # overlap the final exp instead of delaying it
                        warm_dep = pend[-2][1] if len(pend) > 1 else pt
                        while len(pend) > 1:
                            emit_av(j, *pend.pop(0)[0:2], av, dens)
                        if j == NJ - 1:
                            emit_warm(12, dep=warm_dep)
                    for f in fillers.get((j, js), []):
                        f()
                    if av is not None:
                        while len(pend) > 2:
                            emit_av(j, *pend.pop(0)[0:2], av, dens)
                last_pt = pend[-1][1]
                carry[0] = (j, pend, av, dens, attn)
            finalize(carry[0])
            carry[0] = None
            # keep PE warm through the norm/transpose latency
            emit_warm(24, dep=last_pt)

            # tail: j3 out-projection (qk-granular, chases transposes) + stores
            osb = osb3[0]
            for pair in range(2):
                for np2 in (2 * pair, 2 * pair + 1):
                    emit_outproj_np2(NJ - 1, np2, osb, act=True, by_qk=True)
                emit_store(NJ - 1, osb, half=pair)

    nc.compile()
    return nc


def make_in_maps(queries, keys, values, Wq, bq, Wk, bk, Wv, bv, Wo, bo):
    """Build per-core input maps. core = b*4 + g."""
    f32 = np.float32
    x_dt = np.float16
    in_maps = []
    for core in range(8):
        b, g = core // 4, core % 4
        cols = slice(g * EPC, (g + 1) * EPC)
        off = 2 ** g
        ramp = (np.arange(P) - off).astype(f32)
        ramp_last = ramp.copy()
        ramp_last[P - 1] = 1e9
        in_maps.append(
            {
                "xqT": np.ascontiguousarray(queries[b].T).astype(x_dt),
                "xkT": np.ascontiguousarray(keys[b].T).astype(x_dt),
                "xvT": np.ascontiguousarray(values[b].T).astype(x_dt),
                "wqT": np.ascontiguousarray(Wq[cols, :].T).astype(x_dt),
                "wkT": np.ascontiguousarray(Wk[cols, :].T).astype(x_dt),
                "wvT": np.ascontiguousarray(Wv[cols, :].T).astype(x_dt),
                "woT": np.ascontiguousarray(Wo[:, cols].T).astype(x_dt),
                "bqk": np.stack(
                    [bq[cols].reshape(EC, P), bk[cols].reshape(EC, P)]
                ).astype(f32),
                "ramps": np.stack([ramp, ramp_last]),
            }
        )
    return in_maps


def gather_outputs(results, Wo, bv, bo):
    """results: list of 8 dicts with 'outT' [D, L]. Returns [2, L, D]."""
    host_bias = (Wo.astype(np.float64) @ bv.astype(np.float64) + bo).astype(
        np.float32
    )
    out = np.zeros((2, L, D), np.float32)
    for b in range(2):
        acc = np.zeros((D, L), np.float32)
        for g in range(4):
            acc += results[b * 4 + g]["outT"].astype(np.float32)
        out[b] = acc.T + host_bias[None, :]
    return out


# ======================= host entry point =======================
_NC_CACHE = None


def kernel(queries, keys, values, Wq, bq, Wk, bk, Wv, bv, Wo, bo):
    """Full-input entry: shards across 8 NeuronCores, returns [2, 2048, 1024]."""
    global _NC_CACHE
    from concourse.bass_utils import run_bass_kernel_spmd

    args = [np.asarray(a) for a in (queries, keys, values, Wq, bq, Wk, bk, Wv, bv, Wo, bo)]
    queries, keys, values, Wq, bq, Wk, bk, Wv, bv, Wo, bo = args
    if _NC_CACHE is None:
        _NC_CACHE = build_nc()
    in_maps = make_in_maps(queries, keys, values, Wq, bq, Wk, bk, Wv, bv, Wo, bo)
    res = run_bass_kernel_spmd(_NC_CACHE, in_maps, list(range(8)))
    return gather_outputs(res.results, Wo, bv, bo)


# revision 56
# speedup vs baseline: 1.0285x; 1.0285x over previous
"""Sparse diag-masked multi-head attention layer on 8 trn2 cores.

Sharding: core = b*4 + g  (b in 0..1 batches, g in 0..3 head-groups).
Each core computes heads 4g..4g+3 of batch b. Head-group g has band
offset off = 2**g: visible(q, s) <=> s >= q + off  OR  s == L-1.

v3 design (fp16 activations, transposed AV):
  qT, kT   [e 128, ec 2, L]  sbuf fp16    (e on partitions, ec = head pair)
  scoresT  [s 128, 2h, 512]  psum f32 = kT_chunk.T @ qT_chunk (contract e=64)
  pt       [s 128, 4h, nv]   sbuf fp16 = exp(0.125 * scores), band-masked
  AV transposed: av[q 128, t=qk*4+h, e 64] psum += pt_tile.T @ v  (contract s)
     + den[q, t] psum += pt_tile.T @ ones   (1-col matmuls)
     psum zero-region (bank) shared by 8 av tiles: first tile of each bank
     issues start=True (clears the whole bank's has_written bits); the rest
     overwrite into pending-zero bytes; stop on the bank's last touch.
  normalize: attn[q, t, e] = av * reciprocal(den)  (DVE, per-partition scalars)
  aT [e, ec, q] = DMA-transpose(attn)   (XBAR, fp16)
  outT [n, q] psum = woT_chunk.T @ aT_chunk  (contract e=256)
Host: out[b] = sum_g outT(b,g).T + (Wo @ bv + bo)
(bk/bq applied on device during psum->sbuf copies; bv/bo folded on host.)

Schedule: k/q projections for the first score tiles run ec-split so the
first exp lands ~9us in; v/k/q projection chunks and the previous j's
out-projection are woven into the attention js-loops as PE filler.
"""
import sys

sys.path.insert(0, "/opt/trn_rl_repo")

import numpy as np

import concourse.bacc as bacc
import concourse.bass as bass
import concourse.mybir as mybir
import concourse.tile as tile

P = 128
L = 2048
D = 1024
EPC = 256  # head-dims per core (4 heads x 64)
EC = 2  # e-chunks of 128
HPC = 4  # heads per core
NQ = 512  # q-chunk width
NJ = L // NQ  # 4
NSB = L // P  # 16 s-blocks
NDC = D // P  # 8 d-chunks
SCALE = 0.125  # 1/sqrt(64)

F32 = mybir.dt.float32
F16 = mybir.dt.float16
X_DT = F16


def build_nc():
    nc = bacc.Bacc("TRN2", target_bir_lowering=False, debug=False)

    xq = nc.dram_tensor("xqT", [D, L], X_DT, kind="ExternalInput")
    xk = nc.dram_tensor("xkT", [D, L], X_DT, kind="ExternalInput")
    xv = nc.dram_tensor("xvT", [D, L], X_DT, kind="ExternalInput")
    wq = nc.dram_tensor("wqT", [D, EPC], X_DT, kind="ExternalInput")
    wk = nc.dram_tensor("wkT", [D, EPC], X_DT, kind="ExternalInput")
    wv = nc.dram_tensor("wvT", [D, EPC], X_DT, kind="ExternalInput")
    wo = nc.dram_tensor("woT", [EPC, D], X_DT, kind="ExternalInput")
    bqk = nc.dram_tensor("bqk", [2, EC, P], F32, kind="ExternalInput")
    ramps = nc.dram_tensor("ramps", [2, P], F32, kind="ExternalInput")
    out = nc.dram_tensor("outT", [D, L], X_DT, kind="ExternalOutput")

    with tile.TileContext(nc) as tc:
        with (
            tc.tile_pool(name="consts", bufs=1) as consts,
            tc.tile_pool(name="acts", bufs=1) as acts,
            tc.tile_pool(name="xp", bufs=12) as xp,
            tc.tile_pool(name="ptp", bufs=8) as ptp,
            tc.tile_pool(name="attnp", bufs=3) as attnp,
            tc.tile_pool(name="rdp", bufs=2) as rdp,
            tc.tile_pool(name="ttp", bufs=4) as ttp,
            tc.tile_pool(name="osbp", bufs=2) as osbp,
            tc.tile_pool(name="atp", bufs=3) as atp,
            tc.tile_pool(name="scp", bufs=2, space=bass.MemorySpace.PSUM) as scp,
            tc.tile_pool(name="avp", bufs=1, space=bass.MemorySpace.PSUM) as avp,
            tc.tile_pool(name="vpp", bufs=1, space=bass.MemorySpace.PSUM) as vpp,
        ):
            # ---- constants / weights ----
            wk_sb = consts.tile([P, NDC, EPC], X_DT, tag="wk")
            wq_sb = consts.tile([P, NDC, EPC], X_DT, tag="wq")
            wv_sb = consts.tile([P, NDC, EPC], X_DT, tag="wv")
            wo_sb = consts.tile([P, EC, D], X_DT, tag="wo")
            bqk_sb = consts.tile([P, 2, EC], F32, tag="bqk")
            ramps_sb = consts.tile([P, 2], F32, tag="ramps")
            ones_col = consts.tile([P, 1], X_DT, tag="ones_col")
            junk = consts.tile([P, EPC], X_DT, tag="junk")
            c0i = consts.tile([P, NQ], mybir.dt.int32, tag="c0i")
            c0s = consts.tile([P, NQ], X_DT, tag="c0s")

            # ---- persistent activations ----
            qT = acts.tile([P, EC, L], X_DT, tag="qT")
            kT = acts.tile([P, EC, L], X_DT, tag="kT")
            vpk = acts.tile([P, NSB, HPC, 64], X_DT, tag="vpk")
            aT_tiles = {}

            # ---- small on-device consts (issued early; cheap) ----
            nc.vector.memset(junk[:], 0.125)
            nc.vector.memset(ones_col[:], 1.0)
            nc.gpsimd.iota(c0i[:], pattern=[[1, NQ]], base=0, channel_multiplier=0)
            nc.vector.tensor_copy(c0s[:], c0i[:])

            # ---- PE warmup (junk matmuls; output never read) ----
            wu = vpp.tile([P, EC, EPC], F32, tag="vp", name="warm")
            for _ in range(10):
                nc.tensor.matmul(wu[:, 0, :], junk[:, 0:P], junk[:, 0:EPC], start=True, stop=True)

            # ---- x-tile DMAs ([P, dc, q] per 512-col chunk) ----
            xk_t = [None] * NJ
            xq_t = [None] * NJ
            xv_t = [None] * NJ

            def load_x(dst_list, dram, c, tag, halves=False):
                t = xp.tile([P, NDC, NQ], X_DT, tag=tag, name=f"{tag}{c}", bufs=4)
                if halves:
                    for h in range(2):
                        nc.sync.dma_start(
                            t[:, 4 * h : 4 * h + 4, :],
                            dram[
                                4 * h * P : 4 * (h + 1) * P, c * NQ : (c + 1) * NQ
                            ].rearrange("(dc p) q -> p dc q", p=P),
                        )
                else:
                    nc.sync.dma_start(
                        t[:],
                        dram[:, c * NQ : (c + 1) * NQ].rearrange(
                            "(dc p) q -> p dc q", p=P
                        ),
                    )
                dst_list[c] = t

            # DMA issue order = arrival priority.
            def load_x_halves(dst_list, dram, c, tag):
                t = xp.tile([P, NDC, NQ], X_DT, tag=tag, name=f"{tag}{c}", bufs=4)
                for h in range(2):
                    nc.sync.dma_start(
                        t[:, 4 * h : 4 * h + 4, :],
                        dram[
                            4 * h * P : 4 * (h + 1) * P, c * NQ : (c + 1) * NQ
                        ].rearrange("(dc p) q -> p dc q", p=P),
                    )
                dst_list[c] = t

            nc.sync.dma_start(wk_sb[:], wk.rearrange("(dc p) e -> p dc e", p=P))
            load_x_halves(xk_t, xk, 3, "xk")
            nc.sync.dma_start(bqk_sb[:], bqk.rearrange("t c p -> p t c"))
            nc.sync.dma_start(ramps_sb[:], ramps.rearrange("t p -> p t"))
            nc.sync.dma_start(wq_sb[:], wq.rearrange("(dc p) e -> p dc e", p=P))
            load_x_halves(xq_t, xq, 0, "xq")
            nc.sync.dma_start(wv_sb[:], wv.rearrange("(dc p) e -> p dc e", p=P))
            load_x(xv_t, xv, 3, "xv")
            load_x(xk_t, xk, 2, "xk")
            load_x(xv_t, xv, 2, "xv")
            load_x(xk_t, xk, 1, "xk")
            load_x(xq_t, xq, 1, "xq")
            load_x(xv_t, xv, 1, "xv")
            load_x(xk_t, xk, 0, "xk")
            load_x(xv_t, xv, 0, "xv")
            load_x(xq_t, xq, 2, "xq")
            load_x(xq_t, xq, 3, "xq")
            nc.sync.dma_start(wo_sb[:], wo.rearrange("(c p) n -> p c n", p=P))

            # ================= emitters =================
            def emit_kqproj(c, wsb, xc, dst, bias_row, ecs=(0, 1)):
                """Project x chunk c -> dst[:, ec, c*NQ:(c+1)*NQ] (+bias)."""
                slot = scp.tile([P, EC, NQ], F32, tag="sc", name=f"pj{c}")
                for ec in ecs:
                    for dc in range(NDC):
                        nc.tensor.matmul(
                            slot[:, ec, :],
                            wsb[:, dc, ec * P : (ec + 1) * P],
                            xc[:, dc, :],
                            start=(dc == 0),
                            stop=(dc == NDC - 1),
                        )
                for ec in ecs:
                    nc.vector.tensor_scalar_add(
                        dst[:, ec, c * NQ : (c + 1) * NQ],
                        slot[:, ec, :],
                        bqk_sb[:, bias_row, ec : ec + 1],
                    )

            def emit_vproj(pair):
                """Project v s-blocks (2*pair, 2*pair+1) -> vpk."""
                sb0 = 2 * pair
                c, base = sb0 // 4, (sb0 % 4) * P
                xc = xv_t[c]
                slot = vpp.tile([P, EC, EPC], F32, tag="vp", name=f"v{pair}")
                for i in range(2):
                    for dc in range(NDC):
                        nc.tensor.matmul(
                            slot[:, i, :],
                            xc[:, dc, base + i * P : base + (i + 1) * P],
                            wv_sb[:, dc, :],
                            start=(dc == 0),
                            stop=(dc == NDC - 1),
                        )
                nc.vector.tensor_copy(
                    vpk[:, sb0 : sb0 + 2, :, :],
                    slot[:].rearrange("p i (h e) -> p i h e", e=64),
                )

            def emit_scores_pr(j, js, pr, pt, tt):
                """One head-pair of scores + exp + mask for (j, js)."""
                q0 = j * NQ
                s0 = js * P
                k = js - 4 * j
                nv = min(NQ, (k + 1) * P)
                masked = k <= 4
                sp = scp.tile([P, EC, NQ], F32, tag="sc", name=f"sc{js}")
                for i in range(2):
                    nc.tensor.matmul(
                        sp[:, i, 0:nv],
                        kT[i * 64 : i * 64 + 64, pr, s0 : s0 + P],
                        qT[i * 64 : i * 64 + 64, pr, q0 : q0 + nv],
                        start=True,
                        stop=True,
                    )
                nc.scalar.activation(
                    pt[:, 2 * pr : 2 * pr + 2, 0:nv],
                    sp[:, :, 0:nv],
                    mybir.ActivationFunctionType.Exp,
                    scale=SCALE,
                )
                if masked:
                    m0 = max(0, k * P - 8)
                    nc.vector.scalar_tensor_tensor(
                        pt[:, 2 * pr : 2 * pr + 2, m0:nv],
                        c0s[:, None, m0:nv].broadcast_to([P, 2, nv - m0]),
                        tt[:],
                        pt[:, 2 * pr : 2 * pr + 2, m0:nv],
                        op0=mybir.AluOpType.is_le,
                        op1=mybir.AluOpType.mult,
                    )

            def make_tt(j, js):
                k = js - 4 * j
                if k > 4:
                    return None
                tt = ttp.tile([P, 1], F32, tag="t", name="t")
                ramp = ramps_sb[:, 1:2] if js == NSB - 1 else ramps_sb[:, 0:1]
                nc.vector.tensor_scalar_add(tt[:], ramp, float(js * P - j * NQ))
                return tt

            def emit_scores(j, js):
                pt = ptp.tile([P, HPC, NQ], X_DT, tag="pt", name=f"pt{js}")
                tt = make_tt(j, js)
                for pr in range(2):
                    emit_scores_pr(j, js, pr, pt, tt)
                return pt

            def emit_av(j, js, pt, av, dens):
                """Accumulate av[q, t, e] += pt_tile.T @ v ; den += pt.T @ 1.

                den is split per av-bank (qk pair) so each half can be read
                while the other bank is still accumulating."""
                k = js - 4 * j
                nv = min(NQ, (k + 1) * P)
                nqk = nv // P
                first = js == NSB - 1
                for qk in range(nqk):
                    for h in range(HPC):
                        # t-order (ec, qk, h%2) so the whole attn block
                        # transposes to aT[p, ec, qk, q'] in ONE XBAR DMA
                        t = (h // 2) * 8 + qk * 2 + (h % 2)
                        lhsT = pt[:, h, qk * P : (qk + 1) * P]
                        nc.tensor.matmul(
                            av[:, t, :],
                            lhsT,
                            vpk[:, js, h, :],
                            start=(first and qk == 0 and h % 2 == 0),
                            stop=(k == 0 and qk == 0 and h % 2 == 1),
                            skip_group_check=True,
                        )
                        nc.tensor.matmul(
                            dens[:, t : t + 1],
                            lhsT,
                            ones_col[:],
                            start=(first and h == 0 and qk == 0),
                            stop=(k == 0 and qk == 0 and h == HPC - 1),
                            skip_group_check=True,
                        )

            def emit_norm(j, av, dens, attn):
                """attn[q, t, e] = av * 1/den ; one XBAR transpose into aT_j."""
                rd = rdp.tile([P, NSB], F32, tag="rd", name="rd")
                nc.vector.reciprocal(rd[:], dens[:])
                for hb in range(2):
                    t0 = hb * 8
                    nc.vector.tensor_mul(
                        attn[:, t0 : t0 + 8, :],
                        av[:, t0 : t0 + 8, :],
                        rd[:, t0 : t0 + 8, None].broadcast_to([P, 8, 64]),
                    )
                aT_j = atp.tile([P, EC, 4, P], X_DT, tag="aT", name=f"aT{j}")
                aT_tiles[j] = aT_j
                nc.sync.dma_start(
                    aT_j[:],
                    attn[:].rearrange("p t e -> p (t e)"),
                    transpose=True,
                )

            def emit_outproj_np2(j, np2, osb, act=False, by_qk=False):
                """Out-projection for n-pair np2 of q-chunk j into osb.

                by_qk: 128-col matmuls chasing per-qk transposes; the psum
                bank is shared by the 4 qk groups via pending-zero."""
                aT_j = aT_tiles[j]
                ops = scp.tile([P, EC, NQ], F32, tag="sc", name=f"op{np2}")
                if by_qk:
                    for qk in range(4):
                        for i in range(2):
                            n = np2 * 2 + i
                            for c in range(EC):
                                nc.tensor.matmul(
                                    ops[:, i, qk * P : (qk + 1) * P],
                                    wo_sb[:, c, n * P : (n + 1) * P],
                                    aT_j[:, c, qk, :],
                                    start=(qk == 0 and c == 0),
                                    stop=(qk == 3 and c == EC - 1),
                                    skip_group_check=True,
                                )
                else:
                    for i in range(2):
                        n = np2 * 2 + i
                        for c in range(EC):
                            nc.tensor.matmul(
                                ops[:, i, :],
                                wo_sb[:, c, n * P : (n + 1) * P],
                                aT_j[:, c, :, :].rearrange("p a q -> p (a q)"),
                                start=(c == 0),
                                stop=(c == EC - 1),
                            )
                if act:
                    nc.scalar.copy(osb[:, np2, :, :], ops[:])
                else:
                    nc.vector.tensor_copy(osb[:, np2, :, :], ops[:])

            def emit_outproj_half(j, np2, osb, hb, act=False):
                """Half-width out-projection (qk pair hb) for n-pair np2."""
                aT_j = aT_tiles[j]
                ops = scp.tile([P, EC, NQ], F32, tag="sc", name=f"oph{np2}")
                for i in range(2):
                    n = np2 * 2 + i
                    for qk in (2 * hb, 2 * hb + 1):
                        for c in range(EC):
                            nc.tensor.matmul(
                                ops[:, i, qk * P : (qk + 1) * P],
                                wo_sb[:, c, n * P : (n + 1) * P],
                                aT_j[:, c, qk, :],
                                start=(qk == 2 * hb and c == 0),
                                stop=(qk == 2 * hb + 1 and c == EC - 1),
                                skip_group_check=True,
                            )
                h0 = 2 * hb * P
                if act:
                    nc.scalar.copy(
                        osb[:, np2, :, h0 : h0 + 2 * P], ops[:, :, h0 : h0 + 2 * P]
                    )
                else:
                    nc.vector.tensor_copy(
                        osb[:, np2, :, h0 : h0 + 2 * P], ops[:, :, h0 : h0 + 2 * P]
                    )

            def emit_store(j, osb, half=None):
                q0 = j * NQ
                if half is None:
                    nc.gpsimd.dma_start(
                        out[:, q0 : q0 + NQ].rearrange(
                            "(np i p) q -> p np i q", i=2, p=P
                        ),
                        osb[:],
                    )
                else:
                    # row-pair split: np2 {0,1} or {2,3} (disjoint from the
                    # other pair's osb writes)
                    r0 = half * 2 * EC * P
                    nc.gpsimd.dma_start(
                        out[r0 : r0 + 2 * EC * P, q0 : q0 + NQ].rearrange(
                            "(np i p) q -> p np i q", i=2, p=P
                        ),
                        osb[:, 2 * half : 2 * half + 2, :, :],
                    )

            def emit_warm(n, dep=None):
                """Junk matmuls to keep the PE p-state warm. `dep` (an sbuf
                f16 AP) pins them in schedule order via a read dependency —
                without it Tile hoists them to an arbitrary slot."""
                w = vpp.tile([P, EC, EPC], F32, tag="vp", name="warm2")
                lhs = junk[:, 0:P] if dep is None else dep[:, 0, 0:P]
                rhs = junk[:, 0:P] if dep is None else dep[:, 0, 0:P]
                for _ in range(n):
                    nc.tensor.matmul(w[:, 0, 0:P], lhs, rhs, start=True, stop=True)

            # ================= schedule =================
            pre_pts = {}

            def emit_pre_scores(j, js):
                pre_pts[(j, js)] = emit_scores(j, js)

            # Head: ec-split k3/q0 so the first exp lands early.
            emit_kqproj(3, wk_sb, xk_t[3], kT, 1, ecs=(0,))
            emit_kqproj(0, wq_sb, xq_t[0], qT, 0, ecs=(0,))
            pt15 = ptp.tile([P, HPC, NQ], X_DT, tag="pt", name="pt15")
            tt15 = make_tt(0, NSB - 1)
            emit_scores_pr(0, NSB - 1, 0, pt15, tt15)
            emit_kqproj(3, wk_sb, xk_t[3], kT, 1, ecs=(1,))
            emit_kqproj(0, wq_sb, xq_t[0], qT, 0, ecs=(1,))
            emit_scores_pr(0, NSB - 1, 1, pt15, tt15)
            pre_pts[(0, NSB - 1)] = pt15

            # filler work injected into the j-loops, keyed by (j, js)
            fillers = {
                (0, 14): [lambda: emit_vproj(7)],
                (0, 13): [lambda: emit_vproj(6),
                          lambda: emit_kqproj(2, wk_sb, xk_t[2], kT, 1)],
                (0, 12): [lambda: emit_vproj(5)],
                (0, 11): [lambda: emit_vproj(4)],
                (0, 10): [lambda: emit_kqproj(1, wk_sb, xk_t[1], kT, 1)],
                (0, 9): [lambda: emit_vproj(3)],
                (0, 8): [lambda: emit_vproj(2),
                         lambda: emit_kqproj(1, wq_sb, xq_t[1], qT, 0)],
                (0, 7): [lambda: emit_kqproj(0, wk_sb, xk_t[0], kT, 1)],
                (0, 6): [lambda: emit_pre_scores(1, 15)],
                (0, 5): [lambda: emit_vproj(1)],
                (0, 4): [lambda: emit_pre_scores(1, 14)],
                (0, 3): [lambda: emit_vproj(0)],
                (1, 13): [lambda: emit_kqproj(2, wq_sb, xq_t[2], qT, 0)],
                (1, 6): [lambda: emit_pre_scores(2, 15)],
                (1, 5): [lambda: emit_pre_scores(2, 14)],
                (2, 13): [lambda: emit_kqproj(3, wq_sb, xq_t[3], qT, 0)],
                (2, 9): [lambda: emit_pre_scores(3, 15)],
                (2, 8): [lambda: emit_pre_scores(3, 14)],
            }

            def make_outproj_fillers(jprev, j, js_list):
                """Spread outproj(jprev) + store over j's loop."""
                osb = osbp.tile([P, 4, EC, NQ], X_DT, tag="osb", name=f"osb{jprev}")
                if j == NJ - 1:
                    slots = [12, 12, 12, 12]
                else:
                    slots = (js_list[3:] + [js_list[-1]] * 4)[:4]
                act = jprev >= 2
                for idx, js in enumerate(slots):
                    fillers.setdefault((j, js), []).append(
                        lambda np2=idx: emit_outproj_np2(jprev, np2, osb, act=act)
                    )
                fillers.setdefault((j, slots[3]), []).append(
                    lambda: emit_store(jprev, osb)
                )

            osb3 = [None]
            carry = [None]

            def finalize(c):
                jf, rest, avf, densf, attnf = c
                while rest:
                    js_f, pt_f = rest.pop(0)
                    emit_av(jf, js_f, pt_f, avf, densf)
                emit_norm(jf, avf, densf, attnf)

            last_pt = None
            for j in range(NJ):
                js_list = list(range(NSB - 1, 4 * j - 1, -1))
                if j > 0:
                    make_outproj_fillers(j - 1, j, js_list)
                if j == NJ - 1:
                    osb3[0] = osbp.tile([P, 4, EC, NQ], X_DT, tag="osb", name="osb3")
                av = dens = attn = None
                pend = []
                last_js = js_list[-1]
                for js in js_list:
                    pre = pre_pts.pop((j, js), None)
                    pt = pre if pre is not None else emit_scores(j, js)
                    pend.append((js, pt))
                    if av is None and pre is None:
                        # first live scores emitted: finalize the previous j
                        # (its trailing AVs + norm) behind them, then claim
                        # the av psum tiles
                        if carry[0] is not None:
                            finalize(carry[0])
                            carry[0] = None
                        av = avp.tile([P, NSB, 64], F32, tag="av", name=f"av{j}")
                        dens = avp.tile([P, NSB], F32, tag="den", name=f"den{j}")
                        attn = attnp.tile([P, NSB, 64], X_DT, tag="attn", name=f"at{j}")
                    if js == last_js:
                        # flush all but the current so fillers (prev outproj)
                        # overlap the final exp instead of delaying it
                        warm_dep = pend[-2][1] if len(pend) > 1 else pt
                        while len(pend) > 1:
                            emit_av(j, *pend.pop(0)[0:2], av, dens)
                        if j == NJ - 1:
                            emit_warm(12, dep=warm_dep)
                    for f in fillers.get((j, js), []):
                        f()
                    if av is not None:
                        while len(pend) > 2:
                            emit_av(j, *pend.pop(0)[0:2], av, dens)
                last_pt = pend[-1][1]
                carry[0] = (j, pend, av, dens, attn)
            finalize(carry[0])
            carry[0] = None
            # keep PE warm through the norm/transpose latency
            emit_warm(24, dep=last_pt)

            # tail: j3 out-projection (qk-granular, chases transposes) + stores
            osb = osb3[0]
            for pair in range(2):
                for np2 in (2 * pair, 2 * pair + 1):
                    emit_outproj_np2(NJ - 1, np2, osb, act=True, by_qk=True)
                emit_store(NJ - 1, osb, half=pair)

    nc.compile()
    return nc


def make_in_maps(queries, keys, values, Wq, bq, Wk, bk, Wv, bv, Wo, bo):
    """Build per-core input maps. core = b*4 + g."""
    f32 = np.float32
    x_dt = np.float16
    in_maps = []
    for core in range(8):
        b, g = core // 4, core % 4
        cols = slice(g * EPC, (g + 1) * EPC)
        off = 2 ** g
        ramp = (np.arange(P) - off).astype(f32)
        ramp_last = ramp.copy()
        ramp_last[P - 1] = 1e9
        in_maps.append(
            {
                "xqT": np.ascontiguousarray(queries[b].T).astype(x_dt),
                "xkT": np.ascontiguousarray(keys[b].T).astype(x_dt),
                "xvT": np.ascontiguousarray(values[b].T).astype(x_dt),
                "wqT": np.ascontiguousarray(Wq[cols, :].T).astype(x_dt),
                "wkT": np.ascontiguousarray(Wk[cols, :].T).astype(x_dt),
                "wvT": np.ascontiguousarray(Wv[cols, :].T).astype(x_dt),
                "woT": np.ascontiguousarray(Wo[:, cols].T).astype(x_dt),
                "bqk": np.stack(
                    [bq[cols].reshape(EC, P), bk[cols].reshape(EC, P)]
                ).astype(f32),
                "ramps": np.stack([ramp, ramp_last]),
            }
        )
    return in_maps


def gather_outputs(results, Wo, bv, bo):
    """results: list of 8 dicts with 'outT' [D, L]. Returns [2, L, D]."""
    host_bias = (Wo.astype(np.float64) @ bv.astype(np.float64) + bo).astype(
        np.float32
    )
    out = np.zeros((2, L, D), np.float32)
    for b in range(2):
        acc = np.zeros((D, L), np.float32)
        for g in range(4):
            acc += results[b * 4 + g]["outT"].astype(np.float32)
        out[b] = acc.T + host_bias[None, :]
    return out


# ======================= host entry point =======================
_NC_CACHE = None


def kernel(queries, keys, values, Wq, bq, Wk, bk, Wv, bv, Wo, bo):
    """Full-input entry: shards across 8 NeuronCores, returns [2, 2048, 1024]."""
    global _NC_CACHE
    from concourse.bass_utils import run_bass_kernel_spmd

    args = [np.asarray(a) for a in (queries, keys, values, Wq, bq, Wk, bk, Wv, bv, Wo, bo)]
    queries, keys, values, Wq, bq, Wk, bk, Wv, bv, Wo, bo = args
    if _NC_CACHE is None:
        _NC_CACHE = build_nc()
    in_maps = make_in_maps(queries, keys, values, Wq, bq, Wk, bk, Wv, bv, Wo, bo)
    res = run_bass_kernel_spmd(_NC_CACHE, in_maps, list(range(8)))
    return gather_outputs(res.results, Wo, bv, bo)
